# revision 30
# baseline (speedup 1.0000x reference)
"""CSPN propagation step on 8 Trainium2 NeuronCores (pure batch data-parallel).

Math (algebraic collapse of the reference's fold(unfold) structure):
  abs_sum = sum_c |aff_c|;  r = 1/abs_sum;  n_c = aff_c * r
  S[y,x]  = sum_c n_c[y+rho_c, x+delta_c]   (zero outside the image)
  raw_sum = abs_sum * sum_c n_c             (reconstruction, saves a staging pass)
  out     = cur * S + (1 - raw_sum) * coa
with per-channel tap offsets
  c:      0        1       2        3       4        5        6       7
  (rho,d) (+1,+1) (+1,0)  (+1,-1)  (0,+1)  (0,-1)  (-1,+1)  (-1,0)  (-1,-1)

Per core: 8 images, processed in 4 rounds of 2. Layout: partitions = y within a
128-row half, free = [img*half block][x padded to 258], tiles split per channel
PAIR so per-channel chains pipeline (Tile tracks deps per tile). Row shifts and
channel reductions are shifted-/signed-identity fp32r matmuls accumulating in
PSUM (x shifts fold into AP column offsets); |aff| staging on the scalar engine;
normalize + epilogue on vector/gpsimd. The (1-raw_sum)*coa term is accumulated
into the S PSUM bank by one extra matmul per image so the final output is a
single cur*S' multiply. fp32r matmul operands must be written by a compute op
(BIR verifier), hence ACT/DVE/Pool produce all matmul inputs.
"""

import sys

sys.path.insert(0, "/opt/trn_rl_repo")

import numpy as np

from concourse import bass, bacc, mybir, tile
from concourse.bass_utils import run_bass_kernel_spmd

F32 = mybir.dt.float32
F32R = mybir.dt.float32r
ABS = mybir.ActivationFunctionType.Abs
COPY = mybir.ActivationFunctionType.Copy
H = W = 256
PB = 8  # images per core
IPR = 1  # images per round
NROUNDS = PB // IPR
WPAD = W + 2
NBLK = 2 * IPR  # (img, half) blocks per round

# channel -> (row read offset rho, x read offset delta)
TAPS = {0: (1, 1), 1: (1, 0), 2: (1, -1), 3: (0, 1), 4: (0, -1),
        5: (-1, 1), 6: (-1, 0), 7: (-1, -1)}

# stationary-weight indices in the wmats input
W_NI0, W_I0, W_IP1, W_IM1, W_ETF, W_EBF = range(6)

POOL_MUL_CH = (3, 4)  # normalize muls routed to gpsimd


def _wmats_np() -> np.ndarray:
    """[128, 6, 128] stationary matrices, indexed [k, which, m]; out[m] += W[k,m]*X[k]."""
    I = np.eye(128, dtype=np.float32)
    ip1 = np.eye(128, k=-1, dtype=np.float32)  # ones at [m+1, m]: out[m] += X[m+1]
    im1 = np.eye(128, k=1, dtype=np.float32)   # ones at [m-1, m]: out[m] += X[m-1]
    etf = np.zeros((128, 128), np.float32)
    etf[0, 127] = 1.0                          # out[127] += X[0]  (top-half fixup)
    ebf = np.zeros((128, 128), np.float32)
    ebf[127, 0] = 1.0                          # out[0] += X[127]  (bottom-half fixup)
    return np.stack([-I, I, ip1, im1, etf, ebf], axis=0).transpose(1, 0, 2).copy()


def build_program():
    nc = bacc.Bacc("TRN2", target_bir_lowering=False, debug=False)

    aff_d = nc.dram_tensor("affinity", [PB, 8, H, W], F32, kind="ExternalInput").ap()
    cur_d = nc.dram_tensor("cur", [PB, 1, H, W], F32, kind="ExternalInput").ap()
    coa_d = nc.dram_tensor("coa", [PB, 1, H, W], F32, kind="ExternalInput").ap()
    wm_d = nc.dram_tensor("wmats", [128, 6, 128], F32, kind="ExternalInput").ap()
    out_d = nc.dram_tensor("out", [PB, 1, H, W], F32, kind="ExternalOutput").ap()

    with tile.TileContext(nc) as tc:
        with (
            tc.tile_pool(name="wpool", bufs=1) as wpool,
            tc.tile_pool(name="affp", bufs=3) as affp,
            tc.tile_pool(name="npool", bufs=2) as npool,
            tc.tile_pool(name="absp", bufs=3) as absp,
            tc.tile_pool(name="rp", bufs=2) as rp,
            tc.tile_pool(name="segp", bufs=3) as segp,
            tc.tile_pool(name="outp", bufs=2) as outp,
            tc.tile_pool(name="psum", bufs=1, space="PSUM") as psp,
        ):
            wt = wpool.tile([128, 6, 128], F32)
            nc.sync.dma_start(out=wt[:], in_=wm_d[:])
            # fp32r matmul operands must be produced rounded -> round once on ACT
            wtr = wpool.tile([128, 6, 128], F32R)
            nc.scalar.activation(out=wtr[:], in_=wt[:], func=COPY)

            def mm(out_ap, widx, x_ap, start, stop):
                nc.tensor.matmul(
                    out=out_ap,
                    lhsT=wtr[:, widx, :],
                    rhs=x_ap,
                    start=start,
                    stop=stop,
                )

            for rnd in range(NROUNDS):
                b0 = rnd * IPR
                # per-channel-pair tiles: pair p holds channels (2p, 2p+1)
                afft = [affp.tile([128, 2, NBLK, WPAD], F32, tag=f"aff{p}",
                                  name=f"aff{p}_{rnd}") for p in range(4)]
                ntile = [npool.tile([128, 2, NBLK, WPAD], F32R, tag=f"n{p}",
                                    name=f"n{p}_{rnd}") for p in range(4)]
                curt = segp.tile([128, NBLK, W], F32, tag="cur", name=f"cur_{rnd}")
                coat = segp.tile([128, NBLK, W], F32, tag="coa", name=f"coa_{rnd}")
                rt = rp.tile([128, NBLK, WPAD], F32, tag="r", name=f"r_{rnd}")
                abs_sb = rp.tile([128, NBLK, W], F32, tag="abs_sb",
                                 name=f"abs_sb_{rnd}", bufs=2)
                vt = rp.tile([128, NBLK, W], F32, tag="vt", name=f"vt_{rnd}", bufs=2)
                tmp = outp.tile([128, NBLK, W], F32, tag="tmp", name=f"tmp_{rnd}",
                                bufs=2)
                ot = outp.tile([128, NBLK, W], F32, tag="out", name=f"ot_{rnd}", bufs=3)
                nqt = [psp.tile([128, 2, W], F32, tag=f"nq{i}", name=f"nq{i}_{rnd}",
                                bufs=2) for i in range(IPR)]
                abst = [psp.tile([128, 2, W], F32, tag=f"abs{i}", name=f"abs{i}_{rnd}", bufs=2)
                        for i in range(IPR)]
                St = [psp.tile([128, 2, W], F32, tag=f"S{i}", name=f"S{i}_{rnd}", bufs=2)
                      for i in range(IPR)]

                def ch(c):  # (pair tile index, channel-within-pair)
                    return c // 2, c % 2

                # ---- loads: per (pair, img, half) so consumer chains pipeline ----
                for p in range(4):
                    for i in range(IPR):
                        b = b0 + i
                        for h in range(2):
                            nc.sync.dma_start(
                                out=afft[p][:, :, 2 * i + h, 1 : 1 + W],
                                in_=aff_d[b, 2 * p : 2 * p + 2,
                                          128 * h : 128 * (h + 1), :].rearrange(
                                    "c p x -> p c x"
                                ),
                            )
                nc.sync.dma_start(
                    out=curt[:],
                    in_=cur_d[b0 : b0 + IPR, 0].rearrange("b (h p) x -> p (b h) x", p=128),
                )
                nc.sync.dma_start(
                    out=coat[:],
                    in_=coa_d[b0 : b0 + IPR, 0].rearrange("b (h p) x -> p (b h) x", p=128),
                )
                # zero x-pad columns of aff and r: the full-width normalize mul
                # then writes every fp32r byte of n (pads 0*0=0)
                for p in range(4):
                    nc.gpsimd.memset(afft[p][:, :, :, 0 : WPAD : WPAD - 1], 0.0)
                nc.gpsimd.memset(rt[:, :, 0 : WPAD : WPAD - 1], 0.0)

                # ---- abs staging (one op per channel pair, ACT + Pool split)
                # ---- + abs_sum (PSUM) ----
                abtiles = []
                for p in range(4):
                    ab = absp.tile([128, 2, NBLK, W], F32R, tag="ab", name=f"ab{rnd}_{p}")
                    nc.scalar.activation(out=ab[:], in_=afft[p][:, :, :, 1 : 1 + W],
                                         func=ABS)
                    abtiles.append(ab)
                for c in range(8):
                    p, q = ch(c)
                    for i in range(IPR):
                        mm(abst[i][:], W_I0, abtiles[p][:, q, 2 * i : 2 * i + 2, :],
                           start=(c == 0), stop=(c == 7))

                # ---- r = 1/abs_sum; stash abs_sum to SBUF for the epilogue ----
                for i in range(IPR):
                    nc.vector.reciprocal_approx_fast(
                        out=rt[:, 2 * i : 2 * i + 2, 1 : 1 + W], in_=abst[i][:]
                    )
                    nc.scalar.activation(
                        out=abs_sb[:, 2 * i : 2 * i + 2, :], in_=abst[i][:], func=COPY
                    )

                # ---- n_c = aff_c * r (fp32r rounded on write) ----
                for c in range(8):
                    p, q = ch(c)
                    eng = nc.gpsimd if c in POOL_MUL_CH else nc.vector
                    eng.tensor_mul(
                        out=ntile[p][:, q, :, :],
                        in0=afft[p][:, q, :, :],
                        in1=rt[:],
                    )

                # ---- nq = -sum_c n_c (raw_sum = abs_sum * -nq) ----
                for c in range(8):
                    p, q = ch(c)
                    for i in range(IPR):
                        mm(nqt[i][:], W_NI0,
                           ntile[p][:, q, 2 * i : 2 * i + 2, 1 : 1 + W],
                           start=(c == 0), stop=(c == 7))

                # ---- S: shifted-identity matmuls with PSUM accumulation ----
                wmap = {1: W_IP1, 0: W_I0, -1: W_IM1}
                for rho in (1, 0, -1):
                    for c, (rc, dlt) in TAPS.items():
                        if rc != rho:
                            continue
                        p, q = ch(c)
                        for i in range(IPR):
                            mm(St[i][:], wmap[rho],
                               ntile[p][:, q, 2 * i : 2 * i + 2,
                                        1 + dlt : 1 + dlt + W],
                               start=(rho == 1 and c == 0), stop=False)
                # half-boundary fixups: row 127 of the top half reads row 0 of the
                # bottom half (rho=+1 channels); row 0 of the bottom half reads
                # row 127 of the top half (rho=-1). Image-edge rows get zero.
                for i in range(IPR):
                    for c in (0, 1, 2):
                        p, q = ch(c)
                        dlt = TAPS[c][1]
                        mm(St[i][:, 0, :], W_ETF,
                           ntile[p][:, q, 2 * i + 1, 1 + dlt : 1 + dlt + W],
                           start=False, stop=False)
                    for c in (5, 6, 7):
                        p, q = ch(c)
                        dlt = TAPS[c][1]
                        mm(St[i][:, 1, :], W_EBF,
                           ntile[p][:, q, 2 * i, 1 + dlt : 1 + dlt + W],
                           start=False, stop=(c == 7))

                # ---- epilogue: v = abs_sum*nq = -raw_sum; out = cur*S + (v+1)*coa ----
                for i in range(IPR):
                    sl = slice(2 * i, 2 * i + 2)
                    nc.vector.tensor_mul(out=vt[:, sl, :], in0=abs_sb[:, sl, :],
                                         in1=nqt[i][:])
                    nc.vector.scalar_tensor_tensor(
                        out=tmp[:, sl, :], in0=vt[:, sl, :], scalar=1.0,
                        in1=coat[:, sl, :],
                        op0=mybir.AluOpType.add, op1=mybir.AluOpType.mult,
                    )
                    nc.vector.tensor_mul(out=ot[:, sl, :], in0=curt[:, sl, :],
                                         in1=St[i][:])
                    nc.vector.tensor_add(out=ot[:, sl, :], in0=ot[:, sl, :],
                                         in1=tmp[:, sl, :])

                # ---- store (one DMA per round) ----
                nc.sync.dma_start(
                    out=out_d[b0 : b0 + IPR, 0].rearrange("b (h p) x -> p (b h) x", p=128),
                    in_=ot[:],
                )

    nc.compile()
    return nc


_PROG = None


def _get_prog():
    global _PROG
    if _PROG is None:
        _PROG = build_program()
    return _PROG


_WM = _wmats_np()


def kernel(affinity, current_segmentation, coarse_segmentation):
    affinity = np.ascontiguousarray(np.asarray(affinity, dtype=np.float32))
    cur = np.ascontiguousarray(np.asarray(current_segmentation, dtype=np.float32))
    coa = np.ascontiguousarray(np.asarray(coarse_segmentation, dtype=np.float32))
    B = affinity.shape[0]
    n_cores = 8
    per = B // n_cores
    assert per == PB, f"program built for {PB} images/core, got {per}"

    in_maps = []
    for ci in range(n_cores):
        sl = slice(ci * per, (ci + 1) * per)
        in_maps.append({
            "affinity": affinity[sl],
            "cur": cur[sl],
            "coa": coa[sl],
            "wmats": _WM,
        })
    res = run_bass_kernel_spmd(_get_prog(), in_maps, list(range(n_cores)))
    outs = [np.asarray(res.results[ci]["out"]) for ci in range(n_cores)]
    return np.concatenate(outs, axis=0).astype(np.float32)


# revision 32
# speedup vs baseline: 34305.6560x; 34305.6560x over previous
"""CSPN propagation step on 8 Trainium2 NeuronCores (pure batch data-parallel).

Math (algebraic collapse of the reference's fold(unfold) structure):
  abs_sum = sum_c |aff_c|;  r = 1/abs_sum;  n_c = aff_c * r
  S[y,x]  = sum_c n_c[y+rho_c, x+delta_c]   (zero outside the image)
  raw_sum = abs_sum * sum_c n_c             (reconstruction, saves a staging pass)
  out     = cur * S + (1 - raw_sum) * coa
with per-channel tap offsets
  c:      0        1       2        3       4        5        6       7
  (rho,d) (+1,+1) (+1,0)  (+1,-1)  (0,+1)  (0,-1)  (-1,+1)  (-1,0)  (-1,-1)

Per core: 8 images, processed in 8 single-image rounds (finer pipelining). Layout: partitions = y within a
128-row half, free = [img*half block][x padded to 258], tiles split per channel
PAIR so per-channel chains pipeline (Tile tracks deps per tile). Row shifts and
channel reductions are shifted-/signed-identity fp32r matmuls accumulating in
PSUM (x shifts fold into AP column offsets); |aff| staging on the scalar engine;
normalize + epilogue on vector/gpsimd. fp32r matmul operands must be written
by a compute op (BIR verifier rejects DMA-fed fp32r), hence ACT/DVE produce all
matmul inputs rounded.
"""

import sys

sys.path.insert(0, "/opt/trn_rl_repo")

import numpy as np

from concourse import bass, bacc, mybir, tile
from concourse.bass_utils import run_bass_kernel_spmd

F32 = mybir.dt.float32
F32R = mybir.dt.float32r
ABS = mybir.ActivationFunctionType.Abs
COPY = mybir.ActivationFunctionType.Copy
H = W = 256
PB = 8  # images per core
IPR = 1  # images per round
NROUNDS = PB // IPR
WPAD = W + 2
NBLK = 2 * IPR  # (img, half) blocks per round

# channel -> (row read offset rho, x read offset delta)
TAPS = {0: (1, 1), 1: (1, 0), 2: (1, -1), 3: (0, 1), 4: (0, -1),
        5: (-1, 1), 6: (-1, 0), 7: (-1, -1)}

# stationary-weight indices in the wmats input
W_NI0, W_I0, W_IP1, W_IM1, W_ETF, W_EBF = range(6)

POOL_MUL_CH = (3, 4)  # normalize muls routed to gpsimd


def _wmats_np() -> np.ndarray:
    """[128, 6, 128] stationary matrices, indexed [k, which, m]; out[m] += W[k,m]*X[k]."""
    I = np.eye(128, dtype=np.float32)
    ip1 = np.eye(128, k=-1, dtype=np.float32)  # ones at [m+1, m]: out[m] += X[m+1]
    im1 = np.eye(128, k=1, dtype=np.float32)   # ones at [m-1, m]: out[m] += X[m-1]
    etf = np.zeros((128, 128), np.float32)
    etf[0, 127] = 1.0                          # out[127] += X[0]  (top-half fixup)
    ebf = np.zeros((128, 128), np.float32)
    ebf[127, 0] = 1.0                          # out[0] += X[127]  (bottom-half fixup)
    return np.stack([-I, I, ip1, im1, etf, ebf], axis=0).transpose(1, 0, 2).copy()


def build_program():
    nc = bacc.Bacc("TRN2", target_bir_lowering=False, debug=False)

    aff_d = nc.dram_tensor("affinity", [PB, 8, H, W], F32, kind="ExternalInput").ap()
    cur_d = nc.dram_tensor("cur", [PB, 1, H, W], F32, kind="ExternalInput").ap()
    coa_d = nc.dram_tensor("coa", [PB, 1, H, W], F32, kind="ExternalInput").ap()
    wm_d = nc.dram_tensor("wmats", [128, 6, 128], F32, kind="ExternalInput").ap()
    out_d = nc.dram_tensor("out", [PB, 1, H, W], F32, kind="ExternalOutput").ap()

    with tile.TileContext(nc) as tc:
        with (
            tc.tile_pool(name="wpool", bufs=1) as wpool,
            tc.tile_pool(name="affp", bufs=3) as affp,
            tc.tile_pool(name="npool", bufs=2) as npool,
            tc.tile_pool(name="absp", bufs=3) as absp,
            tc.tile_pool(name="rp", bufs=2) as rp,
            tc.tile_pool(name="segp", bufs=3) as segp,
            tc.tile_pool(name="outp", bufs=2) as outp,
            tc.tile_pool(name="psum", bufs=1, space="PSUM") as psp,
        ):
            wt = wpool.tile([128, 6, 128], F32)
            nc.sync.dma_start(out=wt[:], in_=wm_d[:])
            # fp32r matmul operands must be produced rounded -> round once on ACT
            wtr = wpool.tile([128, 6, 128], F32R)
            nc.scalar.activation(out=wtr[:], in_=wt[:], func=COPY)

            def mm(out_ap, widx, x_ap, start, stop):
                nc.tensor.matmul(
                    out=out_ap,
                    lhsT=wtr[:, widx, :],
                    rhs=x_ap,
                    start=start,
                    stop=stop,
                )

            for rnd in range(NROUNDS):
                b0 = rnd * IPR
                # per-channel-pair tiles: pair p holds channels (2p, 2p+1)
                afft = [affp.tile([128, 2, NBLK, WPAD], F32, tag=f"aff{p}",
                                  name=f"aff{p}_{rnd}") for p in range(4)]
                ntile = [npool.tile([128, 2, NBLK, WPAD], F32R, tag=f"n{p}",
                                    name=f"n{p}_{rnd}") for p in range(4)]
                curt = segp.tile([128, NBLK, W], F32, tag="cur", name=f"cur_{rnd}")
                coat = segp.tile([128, NBLK, W], F32, tag="coa", name=f"coa_{rnd}")
                rt = rp.tile([128, NBLK, WPAD], F32, tag="r", name=f"r_{rnd}")
                abs_sb = rp.tile([128, NBLK, W], F32, tag="abs_sb",
                                 name=f"abs_sb_{rnd}", bufs=2)
                vt = rp.tile([128, NBLK, W], F32, tag="vt", name=f"vt_{rnd}", bufs=2)
                tmp = outp.tile([128, NBLK, W], F32, tag="tmp", name=f"tmp_{rnd}",
                                bufs=2)
                ot = outp.tile([128, NBLK, W], F32, tag="out", name=f"ot_{rnd}", bufs=3)
                nqt = [psp.tile([128, 2, W], F32, tag=f"nq{i}", name=f"nq{i}_{rnd}",
                                bufs=2) for i in range(IPR)]
                abst = [psp.tile([128, 2, W], F32, tag=f"abs{i}", name=f"abs{i}_{rnd}", bufs=2)
                        for i in range(IPR)]
                St = [psp.tile([128, 2, W], F32, tag=f"S{i}", name=f"S{i}_{rnd}", bufs=2)
                      for i in range(IPR)]

                def ch(c):  # (pair tile index, channel-within-pair)
                    return c // 2, c % 2

                # ---- loads: per (pair, img, half) so consumer chains pipeline ----
                for p in range(4):
                    for i in range(IPR):
                        b = b0 + i
                        for h in range(2):
                            nc.sync.dma_start(
                                out=afft[p][:, :, 2 * i + h, 1 : 1 + W],
                                in_=aff_d[b, 2 * p : 2 * p + 2,
                                          128 * h : 128 * (h + 1), :].rearrange(
                                    "c p x -> p c x"
                                ),
                            )
                nc.sync.dma_start(
                    out=curt[:],
                    in_=cur_d[b0 : b0 + IPR, 0].rearrange("b (h p) x -> p (b h) x", p=128),
                )
                nc.sync.dma_start(
                    out=coat[:],
                    in_=coa_d[b0 : b0 + IPR, 0].rearrange("b (h p) x -> p (b h) x", p=128),
                )
                # zero x-pad columns of aff and r: the full-width normalize mul
                # then writes every fp32r byte of n (pads 0*0=0)
                for p in range(4):
                    nc.gpsimd.memset(afft[p][:, :, :, 0 : WPAD : WPAD - 1], 0.0)
                nc.gpsimd.memset(rt[:, :, 0 : WPAD : WPAD - 1], 0.0)

                # ---- abs staging (one op per channel pair, ACT + Pool split)
                # ---- + abs_sum (PSUM) ----
                abtiles = []
                for p in range(4):
                    ab = absp.tile([128, 2, NBLK, W], F32R, tag="ab", name=f"ab{rnd}_{p}")
                    nc.scalar.activation(out=ab[:], in_=afft[p][:, :, :, 1 : 1 + W],
                                         func=ABS)
                    abtiles.append(ab)
                for c in range(8):
                    p, q = ch(c)
                    for i in range(IPR):
                        mm(abst[i][:], W_I0, abtiles[p][:, q, 2 * i : 2 * i + 2, :],
                           start=(c == 0), stop=(c == 7))

                # ---- r = 1/abs_sum; stash abs_sum to SBUF for the epilogue ----
                for i in range(IPR):
                    nc.vector.reciprocal_approx_fast(
                        out=rt[:, 2 * i : 2 * i + 2, 1 : 1 + W], in_=abst[i][:]
                    )
                    nc.scalar.activation(
                        out=abs_sb[:, 2 * i : 2 * i + 2, :], in_=abst[i][:], func=COPY
                    )

                # ---- n_c = aff_c * r (fp32r rounded on write) ----
                for c in range(8):
                    p, q = ch(c)
                    eng = nc.gpsimd if c in POOL_MUL_CH else nc.vector
                    eng.tensor_mul(
                        out=ntile[p][:, q, :, :],
                        in0=afft[p][:, q, :, :],
                        in1=rt[:],
                    )

                # ---- nq = -sum_c n_c (raw_sum = abs_sum * -nq) ----
                for c in range(8):
                    p, q = ch(c)
                    for i in range(IPR):
                        mm(nqt[i][:], W_NI0,
                           ntile[p][:, q, 2 * i : 2 * i + 2, 1 : 1 + W],
                           start=(c == 0), stop=(c == 7))

                # ---- S: shifted-identity matmuls with PSUM accumulation ----
                wmap = {1: W_IP1, 0: W_I0, -1: W_IM1}
                for rho in (1, 0, -1):
                    for c, (rc, dlt) in TAPS.items():
                        if rc != rho:
                            continue
                        p, q = ch(c)
                        for i in range(IPR):
                            mm(St[i][:], wmap[rho],
                               ntile[p][:, q, 2 * i : 2 * i + 2,
                                        1 + dlt : 1 + dlt + W],
                               start=(rho == 1 and c == 0), stop=False)
                # half-boundary fixups: row 127 of the top half reads row 0 of the
                # bottom half (rho=+1 channels); row 0 of the bottom half reads
                # row 127 of the top half (rho=-1). Image-edge rows get zero.
                for i in range(IPR):
                    for c in (0, 1, 2):
                        p, q = ch(c)
                        dlt = TAPS[c][1]
                        mm(St[i][:, 0, :], W_ETF,
                           ntile[p][:, q, 2 * i + 1, 1 + dlt : 1 + dlt + W],
                           start=False, stop=False)
                    for c in (5, 6, 7):
                        p, q = ch(c)
                        dlt = TAPS[c][1]
                        mm(St[i][:, 1, :], W_EBF,
                           ntile[p][:, q, 2 * i, 1 + dlt : 1 + dlt + W],
                           start=False, stop=(c == 7))

                # ---- epilogue: v = abs_sum*nq = -raw_sum; out = cur*S + (v+1)*coa ----
                for i in range(IPR):
                    sl = slice(2 * i, 2 * i + 2)
                    nc.vector.tensor_mul(out=vt[:, sl, :], in0=abs_sb[:, sl, :],
                                         in1=nqt[i][:])
                    nc.vector.scalar_tensor_tensor(
                        out=tmp[:, sl, :], in0=vt[:, sl, :], scalar=1.0,
                        in1=coat[:, sl, :],
                        op0=mybir.AluOpType.add, op1=mybir.AluOpType.mult,
                    )
                    nc.vector.tensor_mul(out=ot[:, sl, :], in0=curt[:, sl, :],
                                         in1=St[i][:])
                    nc.vector.tensor_add(out=ot[:, sl, :], in0=ot[:, sl, :],
                                         in1=tmp[:, sl, :])

                # ---- store (one DMA per round) ----
                nc.sync.dma_start(
                    out=out_d[b0 : b0 + IPR, 0].rearrange("b (h p) x -> p (b h) x", p=128),
                    in_=ot[:],
                )

    nc.compile()
    return nc


_PROG = None


def _get_prog():
    global _PROG
    if _PROG is None:
        _PROG = build_program()
    return _PROG


_WM = _wmats_np()


def kernel(affinity, current_segmentation, coarse_segmentation):
    affinity = np.ascontiguousarray(np.asarray(affinity, dtype=np.float32))
    cur = np.ascontiguousarray(np.asarray(current_segmentation, dtype=np.float32))
    coa = np.ascontiguousarray(np.asarray(coarse_segmentation, dtype=np.float32))
    B = affinity.shape[0]
    n_cores = 8
    per = B // n_cores
    assert per == PB, f"program built for {PB} images/core, got {per}"

    in_maps = []
    for ci in range(n_cores):
        sl = slice(ci * per, (ci + 1) * per)
        in_maps.append({
            "affinity": affinity[sl],
            "cur": cur[sl],
            "coa": coa[sl],
            "wmats": _WM,
        })
    res = run_bass_kernel_spmd(_get_prog(), in_maps, list(range(n_cores)))
    outs = [np.asarray(res.results[ci]["out"]) for ci in range(n_cores)]
    return np.concatenate(outs, axis=0).astype(np.float32)


# revision 45
# speedup vs baseline: 34725.9447x; 1.0123x over previous
"""CSPN propagation step on 8 Trainium2 NeuronCores (pure batch data-parallel).

Math (algebraic collapse of the reference's fold(unfold) structure):
  abs_sum = sum_c |aff_c|;  r = 1/abs_sum;  n_c = aff_c * r
  S[y,x]  = sum_c n_c[y+rho_c, x+delta_c]   (zero outside the image)
  raw_sum = abs_sum * sum_c n_c             (reconstruction, saves a staging pass)
  out     = cur * S + (1 - raw_sum) * coa
with per-channel tap offsets
  c:      0        1       2        3       4        5        6       7
  (rho,d) (+1,+1) (+1,0)  (+1,-1)  (0,+1)  (0,-1)  (-1,+1)  (-1,0)  (-1,-1)

Per core: 8 images, processed in 8 single-image rounds (finer pipelining). Layout: partitions = y within a
128-row half, free = [img*half block][x padded to 258], tiles split per channel
PAIR so per-channel chains pipeline (Tile tracks deps per tile). Row shifts and
channel reductions are shifted-/signed-identity fp32r matmuls accumulating in
PSUM (x shifts fold into AP column offsets); |aff| staging on the scalar engine;
normalize + epilogue on vector/gpsimd. fp32r matmul operands must be written
by a compute op (BIR verifier rejects DMA-fed fp32r), hence ACT/DVE produce all
matmul inputs rounded.
"""

import sys

sys.path.insert(0, "/opt/trn_rl_repo")

import numpy as np

from concourse import bass, bacc, mybir, tile
from concourse.bass_utils import run_bass_kernel_spmd

F32 = mybir.dt.float32
F32R = mybir.dt.float32r
ABS = mybir.ActivationFunctionType.Abs
COPY = mybir.ActivationFunctionType.Copy
H = W = 256
PB = 8  # images per core
IPR = 1  # images per round
NROUNDS = PB // IPR
WPAD = W + 2
NBLK = 2 * IPR  # (img, half) blocks per round

# channel -> (row read offset rho, x read offset delta)
TAPS = {0: (1, 1), 1: (1, 0), 2: (1, -1), 3: (0, 1), 4: (0, -1),
        5: (-1, 1), 6: (-1, 0), 7: (-1, -1)}

# stationary-weight indices in the wmats input
W_NI0, W_I0, W_IP1, W_IM1, W_ETF, W_EBF = range(6)

POOL_MUL_CH = (3, 4)  # normalize muls routed to gpsimd


def _wmats_np() -> np.ndarray:
    """[128, 6, 128] stationary matrices, indexed [k, which, m]; out[m] += W[k,m]*X[k]."""
    I = np.eye(128, dtype=np.float32)
    ip1 = np.eye(128, k=-1, dtype=np.float32)  # ones at [m+1, m]: out[m] += X[m+1]
    im1 = np.eye(128, k=1, dtype=np.float32)   # ones at [m-1, m]: out[m] += X[m-1]
    etf = np.zeros((128, 128), np.float32)
    etf[0, 127] = 1.0                          # out[127] += X[0]  (top-half fixup)
    ebf = np.zeros((128, 128), np.float32)
    ebf[127, 0] = 1.0                          # out[0] += X[127]  (bottom-half fixup)
    return np.stack([-I, I, ip1, im1, etf, ebf], axis=0).transpose(1, 0, 2).copy()


def build_program():
    nc = bacc.Bacc("TRN2", target_bir_lowering=False, debug=False)

    aff_d = nc.dram_tensor("affinity", [PB, 8, H, W], F32, kind="ExternalInput").ap()
    cur_d = nc.dram_tensor("cur", [PB, 1, H, W], F32, kind="ExternalInput").ap()
    coa_d = nc.dram_tensor("coa", [PB, 1, H, W], F32, kind="ExternalInput").ap()
    wm_d = nc.dram_tensor("wmats", [128, 6, 128], F32, kind="ExternalInput").ap()
    out_d = nc.dram_tensor("out", [PB, 1, H, W], F32, kind="ExternalOutput").ap()

    with tile.TileContext(nc) as tc:
        with (
            tc.tile_pool(name="wpool", bufs=1) as wpool,
            tc.tile_pool(name="affp", bufs=3) as affp,
            tc.tile_pool(name="npool", bufs=2) as npool,
            tc.tile_pool(name="absp", bufs=3) as absp,
            tc.tile_pool(name="rp", bufs=2) as rp,
            tc.tile_pool(name="segp", bufs=3) as segp,
            tc.tile_pool(name="outp", bufs=2) as outp,
            tc.tile_pool(name="psum", bufs=1, space="PSUM") as psp,
        ):
            wt = wpool.tile([128, 6, 128], F32)
            nc.sync.dma_start(out=wt[:], in_=wm_d[:])
            # fp32r matmul operands must be produced rounded -> round once on ACT
            wtr = wpool.tile([128, 6, 128], F32R)
            nc.scalar.activation(out=wtr[:], in_=wt[:], func=COPY)

            def mm(out_ap, widx, x_ap, start, stop):
                nc.tensor.matmul(
                    out=out_ap,
                    lhsT=wtr[:, widx, :],
                    rhs=x_ap,
                    start=start,
                    stop=stop,
                )

            for rnd in range(NROUNDS):
                b0 = rnd * IPR
                # per-channel-pair tiles: pair p holds channels (2p, 2p+1)
                afft = [affp.tile([128, 2, NBLK, WPAD], F32, tag=f"aff{p}",
                                  name=f"aff{p}_{rnd}") for p in range(4)]
                ntile = [npool.tile([128, 2, NBLK, WPAD], F32R, tag=f"n{p}",
                                    name=f"n{p}_{rnd}") for p in range(4)]
                curt = segp.tile([128, NBLK, W], F32, tag="cur", name=f"cur_{rnd}")
                coat = segp.tile([128, NBLK, W], F32, tag="coa", name=f"coa_{rnd}")
                rt = rp.tile([128, NBLK, WPAD], F32, tag="r", name=f"r_{rnd}")
                abs_sb = rp.tile([128, NBLK, W], F32, tag="abs_sb",
                                 name=f"abs_sb_{rnd}", bufs=2)
                vt = rp.tile([128, NBLK, W], F32, tag="vt", name=f"vt_{rnd}", bufs=2)
                tmp = outp.tile([128, NBLK, W], F32, tag="tmp", name=f"tmp_{rnd}",
                                bufs=2)
                ot = outp.tile([128, NBLK, W], F32, tag="out", name=f"ot_{rnd}", bufs=3)
                nqt = [psp.tile([128, 2, W], F32, tag=f"nq{i}", name=f"nq{i}_{rnd}",
                                bufs=2) for i in range(IPR)]
                abst = [psp.tile([128, 2, W], F32, tag=f"abs{i}", name=f"abs{i}_{rnd}", bufs=2)
                        for i in range(IPR)]
                St = [psp.tile([128, 2, W], F32, tag=f"S{i}", name=f"S{i}_{rnd}", bufs=2)
                      for i in range(IPR)]

                def ch(c):  # (pair tile index, channel-within-pair)
                    return c // 2, c % 2

                # ---- loads: per (pair, img, half) so consumer chains pipeline ----
                for p in range(4):
                    for i in range(IPR):
                        b = b0 + i
                        for h in range(2):
                            nc.sync.dma_start(
                                out=afft[p][:, :, 2 * i + h, 1 : 1 + W],
                                in_=aff_d[b, 2 * p : 2 * p + 2,
                                          128 * h : 128 * (h + 1), :].rearrange(
                                    "c p x -> p c x"
                                ),
                            )
                nc.sync.dma_start(
                    out=curt[:],
                    in_=cur_d[b0 : b0 + IPR, 0].rearrange("b (h p) x -> p (b h) x", p=128),
                )
                nc.sync.dma_start(
                    out=coat[:],
                    in_=coa_d[b0 : b0 + IPR, 0].rearrange("b (h p) x -> p (b h) x", p=128),
                )
                # zero x-pad columns of aff and r: the full-width normalize mul
                # then writes every fp32r byte of n (pads 0*0=0)
                for p in range(4):
                    nc.gpsimd.memset(afft[p][:, :, :, 0 : WPAD : WPAD - 1], 0.0)
                nc.gpsimd.memset(rt[:, :, 0 : WPAD : WPAD - 1], 0.0)

                # ---- abs staging (one ACT op per channel pair) + abs_sum (PSUM) ----
                abtiles = []
                for p in range(4):
                    ab = absp.tile([128, 2, NBLK, W], F32R, tag="ab", name=f"ab{rnd}_{p}")
                    nc.scalar.activation(out=ab[:], in_=afft[p][:, :, :, 1 : 1 + W],
                                         func=ABS)
                    abtiles.append(ab)
                for c in range(8):
                    p, q = ch(c)
                    for i in range(IPR):
                        mm(abst[i][:], W_I0, abtiles[p][:, q, 2 * i : 2 * i + 2, :],
                           start=(c == 0), stop=(c == 7))

                # ---- r = 1/abs_sum; stash abs_sum to SBUF for the epilogue ----
                for i in range(IPR):
                    nc.vector.reciprocal_approx_fast(
                        out=rt[:, 2 * i : 2 * i + 2, 1 : 1 + W], in_=abst[i][:]
                    )
                    nc.scalar.activation(
                        out=abs_sb[:, 2 * i : 2 * i + 2, :], in_=abst[i][:], func=COPY
                    )

                # ---- n_c = aff_c * r (fp32r rounded on write) ----
                for c in range(8):
                    p, q = ch(c)
                    eng = nc.gpsimd if c in POOL_MUL_CH else nc.vector
                    eng.tensor_mul(
                        out=ntile[p][:, q, :, :],
                        in0=afft[p][:, q, :, :],
                        in1=rt[:],
                    )

                # ---- nq = -sum_c n_c (raw_sum = abs_sum * -nq) ----
                for c in range(8):
                    p, q = ch(c)
                    for i in range(IPR):
                        mm(nqt[i][:], W_NI0,
                           ntile[p][:, q, 2 * i : 2 * i + 2, 1 : 1 + W],
                           start=(c == 0), stop=(c == 7))

                # ---- S: shifted-identity matmuls with PSUM accumulation ----
                wmap = {1: W_IP1, 0: W_I0, -1: W_IM1}
                for rho in (1, 0, -1):
                    for c, (rc, dlt) in TAPS.items():
                        if rc != rho:
                            continue
                        p, q = ch(c)
                        for i in range(IPR):
                            mm(St[i][:], wmap[rho],
                               ntile[p][:, q, 2 * i : 2 * i + 2,
                                        1 + dlt : 1 + dlt + W],
                               start=(rho == 1 and c == 0), stop=False)
                # half-boundary fixups: row 127 of the top half reads row 0 of the
                # bottom half (rho=+1 channels); row 0 of the bottom half reads
                # row 127 of the top half (rho=-1). Image-edge rows get zero.
                for i in range(IPR):
                    for c in (0, 1, 2):
                        p, q = ch(c)
                        dlt = TAPS[c][1]
                        mm(St[i][:, 0, :], W_ETF,
                           ntile[p][:, q, 2 * i + 1, 1 + dlt : 1 + dlt + W],
                           start=False, stop=False)
                    for c in (5, 6, 7):
                        p, q = ch(c)
                        dlt = TAPS[c][1]
                        mm(St[i][:, 1, :], W_EBF,
                           ntile[p][:, q, 2 * i, 1 + dlt : 1 + dlt + W],
                           start=False, stop=(c == 7))

                # ---- epilogue: v = abs_sum*nq = -raw_sum; out = cur*S + (v+1)*coa ----
                for i in range(IPR):
                    sl = slice(2 * i, 2 * i + 2)
                    nc.vector.tensor_mul(out=vt[:, sl, :], in0=abs_sb[:, sl, :],
                                         in1=nqt[i][:])
                    nc.vector.scalar_tensor_tensor(
                        out=tmp[:, sl, :], in0=vt[:, sl, :], scalar=1.0,
                        in1=coat[:, sl, :],
                        op0=mybir.AluOpType.add, op1=mybir.AluOpType.mult,
                    )
                    nc.vector.tensor_mul(out=ot[:, sl, :], in0=curt[:, sl, :],
                                         in1=St[i][:])
                    nc.vector.tensor_add(out=ot[:, sl, :], in0=ot[:, sl, :],
                                         in1=tmp[:, sl, :])

                # ---- store (one DMA per round, via Pool/SWDGE: a store on the
                # SP ring head-of-line blocks later rounds' loads behind the
                # epilogue it waits on) ----
                nc.gpsimd.dma_start(
                    out=out_d[b0 : b0 + IPR, 0].rearrange("b (h p) x -> p (b h) x", p=128),
                    in_=ot[:],
                )

    nc.compile()
    return nc


_PROG = None


def _get_prog():
    global _PROG
    if _PROG is None:
        _PROG = build_program()
    return _PROG


_WM = _wmats_np()


def kernel(affinity, current_segmentation, coarse_segmentation):
    affinity = np.ascontiguousarray(np.asarray(affinity, dtype=np.float32))
    cur = np.ascontiguousarray(np.asarray(current_segmentation, dtype=np.float32))
    coa = np.ascontiguousarray(np.asarray(coarse_segmentation, dtype=np.float32))
    B = affinity.shape[0]
    n_cores = 8
    per = B // n_cores
    assert per == PB, f"program built for {PB} images/core, got {per}"

    in_maps = []
    for ci in range(n_cores):
        sl = slice(ci * per, (ci + 1) * per)
        in_maps.append({
            "affinity": affinity[sl],
            "cur": cur[sl],
            "coa": coa[sl],
            "wmats": _WM,
        })
    res = run_bass_kernel_spmd(_get_prog(), in_maps, list(range(n_cores)))
    outs = [np.asarray(res.results[ci]["out"]) for ci in range(n_cores)]
    return np.concatenate(outs, axis=0).astype(np.float32)


# revision 53
# speedup vs baseline: 35023.0306x; 1.0086x over previous
"""CSPN propagation step on 8 Trainium2 NeuronCores (pure batch data-parallel).

Math (algebraic collapse of the reference's fold(unfold) structure):
  abs_sum = sum_c |aff_c|;  r = 1/abs_sum;  n_c = aff_c * r
  S[y,x]  = sum_c n_c[y+rho_c, x+delta_c]   (zero outside the image)
  raw_sum = abs_sum * sum_c n_c             (reconstruction, saves a staging pass)
  out     = cur * S + (1 - raw_sum) * coa
with per-channel tap offsets
  c:      0        1       2        3       4        5        6       7
  (rho,d) (+1,+1) (+1,0)  (+1,-1)  (0,+1)  (0,-1)  (-1,+1)  (-1,0)  (-1,-1)

Per core: 8 images, processed in 8 single-image rounds (finer pipelining). Layout: partitions = y within a
128-row half, free = [img*half block][x padded to 258], tiles split per channel
PAIR so per-channel chains pipeline (Tile tracks deps per tile). Row shifts and
channel reductions are shifted-/signed-identity fp32r matmuls accumulating in
PSUM (x shifts fold into AP column offsets); |aff| staging on the scalar engine;
normalize + epilogue on vector/gpsimd. fp32r matmul operands must be written
by a compute op (BIR verifier rejects DMA-fed fp32r), hence ACT/DVE produce all
matmul inputs rounded.
"""

import sys

sys.path.insert(0, "/opt/trn_rl_repo")

import numpy as np

from concourse import bass, bacc, mybir, tile
from concourse.bass_utils import run_bass_kernel_spmd

F32 = mybir.dt.float32
F32R = mybir.dt.float32r
ABS = mybir.ActivationFunctionType.Abs
COPY = mybir.ActivationFunctionType.Copy
H = W = 256
PB = 8  # images per core
IPR = 1  # images per round
NROUNDS = PB // IPR
WPAD = W + 2
NBLK = 2 * IPR  # (img, half) blocks per round

# channel -> (row read offset rho, x read offset delta)
TAPS = {0: (1, 1), 1: (1, 0), 2: (1, -1), 3: (0, 1), 4: (0, -1),
        5: (-1, 1), 6: (-1, 0), 7: (-1, -1)}

# stationary-weight indices in the wmats input
W_NI0, W_I0, W_IP1, W_IM1, W_ETF, W_EBF = range(6)

POOL_MUL_CH = (3, 4)  # normalize muls routed to gpsimd


def _wmats_np() -> np.ndarray:
    """[128, 6, 128] stationary matrices, indexed [k, which, m]; out[m] += W[k,m]*X[k]."""
    I = np.eye(128, dtype=np.float32)
    ip1 = np.eye(128, k=-1, dtype=np.float32)  # ones at [m+1, m]: out[m] += X[m+1]
    im1 = np.eye(128, k=1, dtype=np.float32)   # ones at [m-1, m]: out[m] += X[m-1]
    etf = np.zeros((128, 128), np.float32)
    etf[0, 127] = 1.0                          # out[127] += X[0]  (top-half fixup)
    ebf = np.zeros((128, 128), np.float32)
    ebf[127, 0] = 1.0                          # out[0] += X[127]  (bottom-half fixup)
    return np.stack([-I, I, ip1, im1, etf, ebf], axis=0).transpose(1, 0, 2).copy()


def build_program():
    nc = bacc.Bacc("TRN2", target_bir_lowering=False, debug=False)

    aff_d = nc.dram_tensor("affinity", [PB, 8, H, W], F32, kind="ExternalInput").ap()
    cur_d = nc.dram_tensor("cur", [PB, 1, H, W], F32, kind="ExternalInput").ap()
    coa_d = nc.dram_tensor("coa", [PB, 1, H, W], F32, kind="ExternalInput").ap()
    wm_d = nc.dram_tensor("wmats", [128, 6, 128], F32, kind="ExternalInput").ap()
    out_d = nc.dram_tensor("out", [PB, 1, H, W], F32, kind="ExternalOutput").ap()

    with tile.TileContext(nc) as tc:
        with (
            tc.tile_pool(name="wpool", bufs=1) as wpool,
            tc.tile_pool(name="affp", bufs=3) as affp,
            tc.tile_pool(name="npool", bufs=2) as npool,
            tc.tile_pool(name="absp", bufs=3) as absp,
            tc.tile_pool(name="rp", bufs=2) as rp,
            tc.tile_pool(name="segp", bufs=3) as segp,
            tc.tile_pool(name="outp", bufs=2) as outp,
            tc.tile_pool(name="psum", bufs=1, space="PSUM") as psp,
        ):
            wt = wpool.tile([128, 6, 128], F32)
            nc.sync.dma_start(out=wt[:], in_=wm_d[:])
            # fp32r matmul operands must be produced rounded -> round once on ACT
            wtr = wpool.tile([128, 6, 128], F32R)
            nc.scalar.activation(out=wtr[:], in_=wt[:], func=COPY)

            def mm(out_ap, widx, x_ap, start, stop):
                nc.tensor.matmul(
                    out=out_ap,
                    lhsT=wtr[:, widx, :],
                    rhs=x_ap,
                    start=start,
                    stop=stop,
                )

            for rnd in range(NROUNDS):
                b0 = rnd * IPR
                # per-channel-pair tiles: pair p holds channels (2p, 2p+1)
                afft = [affp.tile([128, 2, NBLK, WPAD], F32, tag=f"aff{p}",
                                  name=f"aff{p}_{rnd}") for p in range(4)]
                ntile = [npool.tile([128, 2, NBLK, WPAD], F32R, tag=f"n{p}",
                                    name=f"n{p}_{rnd}") for p in range(4)]
                curt = segp.tile([128, NBLK, W], F32, tag="cur", name=f"cur_{rnd}")
                coat = segp.tile([128, NBLK, W], F32, tag="coa", name=f"coa_{rnd}")
                rt = rp.tile([128, NBLK, WPAD], F32, tag="r", name=f"r_{rnd}")
                abs_sb = rp.tile([128, NBLK, W], F32, tag="abs_sb",
                                 name=f"abs_sb_{rnd}", bufs=2)
                vt = rp.tile([128, NBLK, W], F32, tag="vt", name=f"vt_{rnd}", bufs=2)
                tmp = outp.tile([128, NBLK, W], F32, tag="tmp", name=f"tmp_{rnd}",
                                bufs=2)
                ot = outp.tile([128, NBLK, W], F32, tag="out", name=f"ot_{rnd}", bufs=3)
                nqt = [psp.tile([128, 2, W], F32, tag=f"nq{i}", name=f"nq{i}_{rnd}",
                                bufs=2) for i in range(IPR)]
                abst = [psp.tile([128, 2, W], F32, tag=f"abs{i}", name=f"abs{i}_{rnd}", bufs=2)
                        for i in range(IPR)]
                St = [psp.tile([128, 2, W], F32, tag=f"S{i}", name=f"S{i}_{rnd}", bufs=2)
                      for i in range(IPR)]

                def ch(c):  # (pair tile index, channel-within-pair)
                    return c // 2, c % 2

                # ---- loads: per (pair, img, half) so consumer chains pipeline ----
                for p in range(4):
                    for i in range(IPR):
                        b = b0 + i
                        for h in range(2):
                            nc.sync.dma_start(
                                out=afft[p][:, :, 2 * i + h, 1 : 1 + W],
                                in_=aff_d[b, 2 * p : 2 * p + 2,
                                          128 * h : 128 * (h + 1), :].rearrange(
                                    "c p x -> p c x"
                                ),
                            )
                nc.sync.dma_start(
                    out=curt[:],
                    in_=cur_d[b0 : b0 + IPR, 0].rearrange("b (h p) x -> p (b h) x", p=128),
                )
                nc.sync.dma_start(
                    out=coat[:],
                    in_=coa_d[b0 : b0 + IPR, 0].rearrange("b (h p) x -> p (b h) x", p=128),
                )
                # zero x-pad columns of aff and r: the full-width normalize mul
                # then writes every fp32r byte of n (pads 0*0=0)
                for p in range(4):
                    nc.gpsimd.memset(afft[p][:, :, :, 0 : WPAD : WPAD - 1], 0.0)
                nc.gpsimd.memset(rt[:, :, 0 : WPAD : WPAD - 1], 0.0)

                # ---- abs staging (one ACT op per channel pair) + abs_sum (PSUM) ----
                abtiles = []
                for p in range(4):
                    ab = absp.tile([128, 2, NBLK, W], F32R, tag="ab", name=f"ab{rnd}_{p}")
                    nc.scalar.activation(out=ab[:], in_=afft[p][:, :, :, 1 : 1 + W],
                                         func=ABS)
                    abtiles.append(ab)
                for c in range(8):
                    p, q = ch(c)
                    for i in range(IPR):
                        mm(abst[i][:], W_I0, abtiles[p][:, q, 2 * i : 2 * i + 2, :],
                           start=(c == 0), stop=(c == 7))

                # ---- r = 1/abs_sum; stash abs_sum to SBUF for the epilogue ----
                for i in range(IPR):
                    nc.vector.reciprocal_approx_fast(
                        out=rt[:, 2 * i : 2 * i + 2, 1 : 1 + W], in_=abst[i][:]
                    )
                    nc.scalar.activation(
                        out=abs_sb[:, 2 * i : 2 * i + 2, :], in_=abst[i][:], func=COPY
                    )

                # ---- n_c = aff_c * r (fp32r rounded on write) ----
                for c in range(8):
                    p, q = ch(c)
                    eng = nc.gpsimd if c in POOL_MUL_CH else nc.vector
                    eng.tensor_mul(
                        out=ntile[p][:, q, :, :],
                        in0=afft[p][:, q, :, :],
                        in1=rt[:],
                    )

                # ---- nq = -sum_c n_c (raw_sum = abs_sum * -nq) ----
                for c in range(8):
                    p, q = ch(c)
                    for i in range(IPR):
                        mm(nqt[i][:], W_NI0,
                           ntile[p][:, q, 2 * i : 2 * i + 2, 1 : 1 + W],
                           start=(c == 0), stop=(c == 7))

                # ---- S: shifted-identity matmuls with PSUM accumulation ----
                wmap = {1: W_IP1, 0: W_I0, -1: W_IM1}
                for rho in (1, 0, -1):
                    for c, (rc, dlt) in TAPS.items():
                        if rc != rho:
                            continue
                        p, q = ch(c)
                        for i in range(IPR):
                            mm(St[i][:], wmap[rho],
                               ntile[p][:, q, 2 * i : 2 * i + 2,
                                        1 + dlt : 1 + dlt + W],
                               start=(rho == 1 and c == 0), stop=False)
                # half-boundary fixups: row 127 of the top half reads row 0 of the
                # bottom half (rho=+1 channels); row 0 of the bottom half reads
                # row 127 of the top half (rho=-1). Image-edge rows get zero.
                for i in range(IPR):
                    for c in (0, 1, 2):
                        p, q = ch(c)
                        dlt = TAPS[c][1]
                        mm(St[i][:, 0, :], W_ETF,
                           ntile[p][:, q, 2 * i + 1, 1 + dlt : 1 + dlt + W],
                           start=False, stop=False)
                    for c in (5, 6, 7):
                        p, q = ch(c)
                        dlt = TAPS[c][1]
                        mm(St[i][:, 1, :], W_EBF,
                           ntile[p][:, q, 2 * i, 1 + dlt : 1 + dlt + W],
                           start=False, stop=(c == 7))

                # ---- epilogue: v = abs_sum*nq = -raw_sum; out = cur*S + (v+1)*coa ----
                for i in range(IPR):
                    sl = slice(2 * i, 2 * i + 2)
                    nc.vector.tensor_mul(out=vt[:, sl, :], in0=abs_sb[:, sl, :],
                                         in1=nqt[i][:])
                    nc.vector.scalar_tensor_tensor(
                        out=tmp[:, sl, :], in0=vt[:, sl, :], scalar=1.0,
                        in1=coat[:, sl, :],
                        op0=mybir.AluOpType.add, op1=mybir.AluOpType.mult,
                    )
                    nc.vector.tensor_mul(out=ot[:, sl, :], in0=curt[:, sl, :],
                                         in1=St[i][:])
                    nc.vector.tensor_add(out=ot[:, sl, :], in0=ot[:, sl, :],
                                         in1=tmp[:, sl, :])

                # ---- store (one DMA per round, via Pool/SWDGE: a store on the
                # SP ring head-of-line blocks later rounds' loads behind the
                # epilogue it waits on) ----
                steng = nc.sync if rnd >= NROUNDS - 3 else nc.gpsimd
                steng.dma_start(
                    out=out_d[b0 : b0 + IPR, 0].rearrange("b (h p) x -> p (b h) x", p=128),
                    in_=ot[:],
                )

    nc.compile()
    return nc


_PROG = None


def _get_prog():
    global _PROG
    if _PROG is None:
        _PROG = build_program()
    return _PROG


_WM = _wmats_np()


def kernel(affinity, current_segmentation, coarse_segmentation):
    affinity = np.ascontiguousarray(np.asarray(affinity, dtype=np.float32))
    cur = np.ascontiguousarray(np.asarray(current_segmentation, dtype=np.float32))
    coa = np.ascontiguousarray(np.asarray(coarse_segmentation, dtype=np.float32))
    B = affinity.shape[0]
    n_cores = 8
    per = B // n_cores
    assert per == PB, f"program built for {PB} images/core, got {per}"

    in_maps = []
    for ci in range(n_cores):
        sl = slice(ci * per, (ci + 1) * per)
        in_maps.append({
            "affinity": affinity[sl],
            "cur": cur[sl],
            "coa": coa[sl],
            "wmats": _WM,
        })
    res = run_bass_kernel_spmd(_get_prog(), in_maps, list(range(n_cores)))
    outs = [np.asarray(res.results[ci]["out"]) for ci in range(n_cores)]
    return np.concatenate(outs, axis=0).astype(np.float32)
